# revision 91
# baseline (speedup 1.0000x reference)
"""Trainium2 Bass kernel for Ernie4.5-VL attention (mRoPE + GQA causal attention).

Sharding: tensor-parallel over heads across 8 cores. Each core computes
2 q heads + its kv head (replicated per core pair): qkv projection
(q/k feature-major, V token-major directly — no transposes), interleaved
mRoPE (via a host-side even/odd column permutation of the q/k weight
slices so the rotation becomes two contiguous partition halves), causal
attention with unnormalized softmax (denominator via bf16 tile adds +
one all-ones matmul), and the o_proj partial product. Host sums the 8
partial outputs.

All tensors move through SBUF/DRAM as bf16; matmuls are bf16 in / fp32
psum out; psum evacuations round once to bf16. Schedule: a flat
software pipeline where attention chunk g's score->exp->AV steps are
interleaved (emission-order round-robin) with chunk g+1's projection
matmuls and chunk g-1's o_proj — the PE fills exp (ACT) latency with
projection work instead of stalling, which also keeps the PE p-state
ramped.
"""
import numpy as np
import ml_dtypes
from contextlib import ExitStack

import concourse.bacc as bacc
import concourse.tile as tile
from concourse import mybir
from concourse.bass_utils import run_bass_kernel_spmd

HIDDEN = 2048
T = 2048
N_HEADS = 16
N_KV = 4
HD = 128
THETA = 500000.0
NCORES = 8
SCALE = HD ** -0.5

F32 = mybir.dt.float32
BF16 = mybir.dt.bfloat16
FP8 = mybir.dt.float8e4
I32 = mybir.dt.int32
DR = mybir.MatmulPerfMode.DoubleRow

# within-head column permutation: evens then odds (so interleaved rope pairs
# become two contiguous partition halves in feature-major layout)
PERM = np.concatenate([np.arange(0, HD, 2), np.arange(1, HD, 2)])
# pair index p (0..63): p<44: even->pos row 1 (h), odd->row 2 (w); p>=44: row 0 (t)
ROW_MAP = np.array([(1 if p % 2 == 0 else 2) if p < 44 else 0 for p in range(64)])
INVF = (THETA ** (-(np.arange(64, dtype=np.float64) / 64))).astype(np.float32)

NT = T // 128      # 16 token tiles
NG = T // 512      # 4 token chunks
NH_T = HIDDEN // 128  # 16 hidden tiles

# schedule knobs (tuned via TimelineSim sweep)
KNOBS = {
    "ou_first": False,   # oproj units before proj units in the filler mix
    "rope_add_dve": False,  # rope final add on DVE (False: Pool)
    "ybuf_dve_mod": 0,   # ybuf evac: i % mod == 2 -> DVE, else ACT (0: 50/50)
    "lead": 2,           # fillers emitted before step 0 of each phase
    "xt_at_end": True,   # prefetch xt(g+2) at phase end (False: start)
    "ou_head": 8,        # oproj units mixed in early (rest appended at end)
}


def _build(dbg=False):
    nc = bacc.Bacc("TRN2", target_bir_lowering=False, debug=False)
    d_xh = nc.dram_tensor("xh", [HIDDEN, T], FP8, kind="ExternalInput").ap()
    d_xl = nc.dram_tensor("xl", [HIDDEN, T], FP8, kind="ExternalInput").ap()
    d_wh = nc.dram_tensor("wh", [HIDDEN, 512], FP8, kind="ExternalInput").ap()
    d_wl = nc.dram_tensor("wl", [HIDDEN, 512], FP8, kind="ExternalInput").ap()
    d_woh = nc.dram_tensor("woh", [256, HIDDEN], FP8, kind="ExternalInput").ap()
    d_wol = nc.dram_tensor("wol", [256, HIDDEN], FP8, kind="ExternalInput").ap()
    d_cdup = nc.dram_tensor("cdup", [128, T], BF16, kind="ExternalInput").ap()
    d_sflip = nc.dram_tensor("sflip", [128, T], BF16, kind="ExternalInput").ap()
    d_mL = nc.dram_tensor("mask_l", [128, 128], BF16, kind="ExternalInput").ap()
    d_mR = nc.dram_tensor("mask_rb", [128, 128], BF16, kind="ExternalInput").ap()
    d_ones = nc.dram_tensor("ones", [128, 128], BF16, kind="ExternalInput").ap()
    d_yT = nc.dram_tensor("yT", [HIDDEN, T], BF16, kind="ExternalOutput").ap()
    if dbg:
        d_qkv = nc.dram_tensor("dbg_qkv", [128, 3, T], BF16, kind="ExternalOutput").ap()
        d_V = nc.dram_tensor("dbg_V", [128, NT, 128], BF16, kind="ExternalOutput").ap()
        d_O = nc.dram_tensor("dbg_O", [128, 2, T], FP8, kind="ExternalOutput").ap()

    with tile.TileContext(nc) as tc, ExitStack() as ctx:
        const = ctx.enter_context(tc.tile_pool(name="const", bufs=1))
        big = ctx.enter_context(tc.tile_pool(name="big", bufs=1))

        # resident tiles. qkv projection runs as fp8e4m3 hi+lo residual
        # DoubleRow matmuls (xh@wh + xh@wl + xl@wh = 1.5 half-rate passes,
        # 0.75x the bf16 cost at ~bf16 accuracy).
        wh_sb = const.tile([128, NH_T, 512], FP8)
        wl_sb = const.tile([128, NH_T, 512], FP8)
        woh_sb = const.tile([128, 2, HIDDEN], FP8)      # o_proj rows, hi+lo
        wol_sb = const.tile([128, 2, HIDDEN], FP8)
        mL_sb = const.tile([128, 128], BF16)            # causal mask, left factor
        mR_sb = const.tile([128, 128], BF16)            # causal mask, right factor
        ones_sb = const.tile([128, 128], BF16)
        qkv_sb = big.tile([128, 3, T], BF16)            # q0|q1|k feature-major (roped)
        V_sb = big.tile([128, NT, 128], BF16)           # V token-major
        Oh_sb = big.tile([128, 2, T], FP8)              # attention out, fp8 hi+lo
        Ol_sb = big.tile([128, 2, T], FP8)              # (dim 1 = head = DoubleRow kt)
        cdup = big.tile([128, T], BF16)                 # cos table (dup halves)
        sflip = big.tile([128, T], BF16)                # sin table ([-s; s])

        # PSUM budget (8 banks): projection feature accum 2 (k/q0/q1 rotate —
        # a feature's bank frees once its rope has read it) + V-direct 1 +
        # shared(scores/o_proj) 3 + AV accum 2 (both heads in flight).
        xtp = ctx.enter_context(tc.tile_pool(name="xt", bufs=2))
        qkvp = ctx.enter_context(tc.tile_pool(name="qkvp", bufs=2, space="PSUM"))
        vdp = ctx.enter_context(tc.tile_pool(name="vdp", bufs=1, space="PSUM"))
        spp = ctx.enter_context(tc.tile_pool(name="spp", bufs=3, space="PSUM"))
        avp = ctx.enter_context(tc.tile_pool(name="avp", bufs=2, space="PSUM"))
        rp = ctx.enter_context(tc.tile_pool(name="rope", bufs=2))
        ep = ctx.enter_context(tc.tile_pool(name="ep", bufs=8))
        rv = ctx.enter_context(tc.tile_pool(name="rv", bufs=2))
        racc = ctx.enter_context(tc.tile_pool(name="racc", bufs=2))
        yo = ctx.enter_context(tc.tile_pool(name="yo", bufs=2))

        # ---- startup DMAs, ordered so the first projection matmuls unblock
        # as early as possible (cos/sin rope tables are host-computed)
        xt_tiles = {}

        def load_xt(g):
            th = xtp.tile([128, NH_T, 512], FP8, tag="xth", name=f"xth_{g}")
            tl = xtp.tile([128, NH_T, 512], FP8, tag="xtl", name=f"xtl_{g}")
            xt_tiles[g] = (th, tl)
            for q4 in range(4):
                sl = np.s_[:, 4 * q4:4 * (q4 + 1), :]
                dsl = np.s_[512 * q4:512 * (q4 + 1), 512 * g:512 * (g + 1)]
                nc.sync.dma_start(
                    out=th[sl], in_=d_xh[dsl].rearrange("(a p) c -> p a c", p=128))
                nc.sync.dma_start(
                    out=tl[sl], in_=d_xl[dsl].rearrange("(a p) c -> p a c", p=128))

        xt0h = xtp.tile([128, NH_T, 512], FP8, tag="xth", name="xth_0")
        xt0l = xtp.tile([128, NH_T, 512], FP8, tag="xtl", name="xtl_0")
        xt_tiles[0] = (xt0h, xt0l)
        for lo, hi in ((0, 2), (2, 4), (4, 8), (8, 16)):
            for w_d, w_s, x_d, x_s in ((wh_sb, d_wh, xt0h, d_xh),
                                       (wl_sb, d_wl, xt0l, d_xl)):
                nc.sync.dma_start(
                    out=w_d[:, lo:hi, :],
                    in_=w_s[128 * lo:128 * hi, :].rearrange(
                        "(a p) c -> p a c", p=128))
                nc.sync.dma_start(
                    out=x_d[:, lo:hi, :],
                    in_=x_s[128 * lo:128 * hi, 0:512].rearrange(
                        "(a p) c -> p a c", p=128))
        nc.sync.dma_start(out=cdup, in_=d_cdup)
        nc.sync.dma_start(out=sflip, in_=d_sflip)
        nc.sync.dma_start(out=mL_sb, in_=d_mL)
        nc.sync.dma_start(out=mR_sb, in_=d_mR)
        nc.sync.dma_start(out=ones_sb, in_=d_ones)

        def load_wo():
            # deferred until after rope(0)'s swap DMAs: the FIFO DMA device
            # must not put bulk traffic in front of the latency-critical
            # swaps (wo isn't needed until oproj(0) in phase 1)
            for dst, src in ((woh_sb, d_woh), (wol_sb, d_wol)):
                nc.sync.dma_start(
                    out=dst, in_=src.rearrange("(a p) c -> p a c", p=128))

        # ================= stage emitters =================
        proj_state = {}

        def proj_alloc(g):
            assert g in xt_tiles, f"xt chunk {g} not prefetched"
            proj_state[g] = {
                "ps": {},
                "vd": vdp.tile([128, 4, 128], F32, tag="vd", name=f"vd_{g}"),
            }

        def proj_ps(g, i):
            # lazy per-feature psum: the 2-deep ring reuses a feature's bank
            # only after its rope has consumed it (k -> q0 -> q1 order)
            st = proj_state[g]
            if i not in st["ps"]:
                st["ps"][i] = qkvp.tile([128, 512], F32, tag="qkvps",
                                        name=f"qkvps_{g}_{i}")
            return st["ps"][i]

        NP_ = NH_T // 2   # DoubleRow passes (2 hidden tiles per pass)

        def mk_proj_feat(g, i, us):
            """One feature's (q0/q1/k) psum accumulation over DoubleRow
            passes `us`, three residual terms per pass."""
            def emit():
                if i == 2 and us[0] == 0:
                    proj_alloc(g)
                ps = proj_ps(g, i)
                xh_t, xl_t = xt_tiles[g]
                for u in us:
                    hsl = np.s_[:, 2 * u:2 * (u + 1)]
                    for t, (xa, wb) in enumerate(((xh_t, wh_sb), (xh_t, wl_sb),
                                                  (xl_t, wh_sb))):
                        nc.tensor.matmul(
                            ps[:],
                            wb[:, 2 * u:2 * (u + 1), 128 * i:128 * (i + 1)],
                            xa[hsl],
                            start=(u == 0 and t == 0),
                            stop=(u == NP_ - 1 and t == 2), perf_mode=DR)
            return emit

        def mk_proj_v(g, tt):
            # one token tile's full contraction as a contiguous start..stop
            # group: interleaved accumulation groups at different offsets of
            # the SAME psum bank produce wrong results on hardware
            def emit():
                st = proj_state[g]
                xh_t, xl_t = xt_tiles[g]
                for u in range(NP_):
                    for t, (xa, wb) in enumerate(((xh_t, wh_sb), (xh_t, wl_sb),
                                                  (xl_t, wh_sb))):
                        nc.tensor.matmul(
                            st["vd"][:, tt, :],
                            xa[:, 2 * u:2 * (u + 1), 128 * tt:128 * (tt + 1)],
                            wb[:, 2 * u:2 * (u + 1), 384:512],
                            start=(u == 0 and t == 0),
                            stop=(u == NP_ - 1 and t == 2), perf_mode=DR)
            return emit

        def proj_units_startup(g):
            """Chunk 0: k and q0 pass-major (follows the startup DMA arrival
            order, using both psum ring slots), then their ropes free k's
            bank for q1."""
            units = []

            def mk(u):
                def emit():
                    if u == 0:
                        proj_alloc(g)
                    xh_t, xl_t = xt_tiles[g]
                    for i in (2, 0):
                        ps = proj_ps(g, i)
                        for t, (xa, wb) in enumerate(((xh_t, wh_sb),
                                                      (xh_t, wl_sb),
                                                      (xl_t, wh_sb))):
                            nc.tensor.matmul(
                                ps[:],
                                wb[:, 2 * u:2 * (u + 1), 128 * i:128 * (i + 1)],
                                xa[:, 2 * u:2 * (u + 1), :],
                                start=(u == 0 and t == 0),
                                stop=(u == NP_ - 1 and t == 2), perf_mode=DR)
                return emit

            def late_loads():
                load_xt(1)
                load_wo()

            ru = rope_units(g, add_dve=True)   # DVE is idle at startup
            duos = [list(range(2 * q, 2 * q + 2)) for q in range(4)]
            units = [mk(u) for u in range(NP_)] + [ru[0], ru[2], late_loads]
            units += [mk_proj_v(g, tt) for tt in range(4)] + [ru[1]]
            units += [mk_proj_feat(g, 1, u2) for u2 in duos] + [ru[3]]
            return units

        def proj_units(g):
            """Chunk g's projection + rope as filler units, ordered so each
            feature's psum completes early and its rope follows immediately:
            k first (gates next phase's scores), then q0, V, q1."""
            ru = rope_units(g)
            duos = [list(range(2 * q, 2 * q + 2)) for q in range(4)]
            units = [mk_proj_feat(g, 2, u2) for u2 in duos] + [ru[0]]
            units += [mk_proj_feat(g, 0, u2) for u2 in duos] + [ru[2]]
            units += [mk_proj_v(g, tt) for tt in range(4)] + [ru[1]]
            units += [mk_proj_feat(g, 1, u2) for u2 in duos] + [ru[3]]
            return units

        def rope_units(g, add_dve=None):
            """V evac (Pool) + mRoPE for k/q0/q1 of chunk g as filler units,
            reading the projection psums (swapped halves via a bf16 scratch
            + SBUF->SBUF DMA), writing bf16 qkv_sb once. k first: it gates
            chunk g's scores."""
            tsl = np.s_[512 * g:512 * (g + 1)]
            xs = rp.tile([128, 3, 512], BF16, tag="xs", name=f"xs{g}")

            def mk_rope(t3):
                def emit():
                    st = proj_state[g]
                    psx = st["ps"][t3]
                    x = qkv_sb[:, t3, tsl]
                    xraw = rp.tile([128, 512], BF16, tag="xraw",
                                   name=f"xr_{g}_{t3}")
                    nc.vector.tensor_copy(xraw[:], psx[:])
                    nc.sync.dma_start(out=xs[0:64, t3, :], in_=xraw[64:128, :])
                    nc.sync.dma_start(out=xs[64:128, t3, :], in_=xraw[0:64, :])
                    t1 = rp.tile([128, 512], F32, tag="t1", name=f"t1_{g}_{t3}")
                    t2 = rp.tile([128, 512], F32, tag="t2", name=f"t2_{g}_{t3}")
                    # GPSIMD cannot touch PSUM: t1 (psum read) stays on DVE;
                    # t2 is SBUF-only so Pool takes it; the add returns to
                    # DVE (Pool's ~1.2us ops would serialize the k chain
                    # that gates the next phase's first scores)
                    nc.vector.tensor_mul(t1[:], psx[:], cdup[:, tsl])
                    nc.gpsimd.tensor_mul(t2[:], xs[:, t3, :], sflip[:, tsl])
                    on_dve = KNOBS["rope_add_dve"] if add_dve is None else add_dve
                    if on_dve:
                        nc.vector.tensor_add(x, t1[:], t2[:])
                    else:
                        nc.gpsimd.tensor_add(x, t1[:], t2[:])
                return emit

            def mk_vevac():
                def emit():
                    st = proj_state[g]
                    for tt in range(4):
                        nc.vector.tensor_copy(V_sb[:, 4 * g + tt, :],
                                              st["vd"][:, tt, :])
                return emit

            return [mk_rope(2), mk_vevac(), mk_rope(0), mk_rope(1)]

        def attn_steps(g):
            """Flat list of per-j-step emitters for both heads of chunk g.
            Scores run one step ahead of AV; head-0's denominator tail is
            emitted two steps into head 1 so its latency hides behind
            head-1 scores."""
            tsl = np.s_[512 * g:512 * (g + 1)]
            jmax = 4 * g + 4
            state = {}

            def head_alloc(h):
                state[h] = {
                    "po": avp.tile([128, 512], F32, tag="av", name=f"po{g}_{h}"),
                    "ra": racc.tile([128, 512], BF16, tag="ra", name=f"ra{g}_{h}"),
                    "Es": [None] * jmax,
                }

            # diagonal tiles (m = j-4g >= 0): queries [0:128m) can never see
            # this key tile, so scores/exp/AV/row-sum all trim to [128m:512]
            # and only the 128-wide boundary subtile needs the factored
            # triangular mask. The psum/E region left of 128m stays garbage
            # and is never read.
            def lo_of(j):
                return max(0, 128 * (j - 4 * g))

            def mk_av(h, j, stop):
                lo = lo_of(j)
                st = state[h]
                nc.tensor.matmul(st["po"][:, lo:], V_sb[:, j, :],
                                 st["Es"][j][:, lo:],
                                 start=(j == 0), stop=stop,
                                 skip_group_check=True)

            def mk_step(h, j):
                def emit():
                    if j == 0:
                        head_alloc(h)
                    st = state[h]
                    qc = qkv_sb[:, h, tsl]
                    m = j - 4 * g
                    lo = lo_of(j)
                    ps = spp.tile([128, 512], F32, tag="sp", name=f"s{g}_{h}_{j}")
                    nc.tensor.matmul(ps[:, lo:], qkv_sb[:, 2, 128 * j:128 * (j + 1)],
                                     qc[:, lo:], start=True, stop=(m < 0),
                                     skip_group_check=True)
                    if m >= 0:
                        nc.tensor.matmul(ps[:, lo:lo + 128], mL_sb[:], mR_sb[:],
                                         start=False, stop=True,
                                         skip_group_check=True)
                    E = ep.tile([128, 512], BF16, tag="e", name=f"e{g}_{h}_{j}")
                    st["Es"][j] = E
                    nc.scalar.activation(E[:, lo:], ps[:, lo:],
                                         mybir.ActivationFunctionType.Exp,
                                         scale=SCALE)
                    # row-sum partial accumulation on DVE (in-order engine:
                    # a single chain has the same throughput as two and a
                    # shorter tail)
                    if j == 0:
                        nc.vector.tensor_copy(st["ra"][:], E[:])
                    else:
                        nc.vector.tensor_add(st["ra"][:, lo:], st["ra"][:, lo:],
                                             E[:, lo:])
                    if j >= 1:
                        mk_av(h, j - 1, stop=False)
                return emit

            def mk_tail(h):
                def emit():
                    st = state[h]
                    mk_av(h, jmax - 1, stop=True)
                    # r broadcast across partitions via one all-ones matmul
                    # (ones carries x64 so rinv folds the fp8 V/O scaling)
                    pr = spp.tile([128, 512], F32, tag="sp", name=f"pr{g}_{h}")
                    nc.tensor.matmul(pr[:], ones_sb[:], st["ra"][:],
                                     start=True, stop=True)
                    rinv = rv.tile([128, 512], F32, tag="rv", name=f"rinv{g}_{h}")
                    nc.vector.reciprocal(rinv[:], pr[:])
                    # O = po*rinv split into fp8 hi+lo across DVE/ACT/Pool
                    # (Pool may not touch psum, so the f32 product lands in
                    # SBUF first)
                    t = rv.tile([128, 512], F32, tag="ot", name=f"ot{g}_{h}")
                    nc.vector.tensor_mul(t[:], st["po"][:], rinv[:])
                    nc.scalar.activation(Oh_sb[:, h, tsl], t[:],
                                         mybir.ActivationFunctionType.Copy)
                    nc.gpsimd.tensor_sub(Ol_sb[:, h, tsl], t[:],
                                         Oh_sb[:, h, tsl])
                return emit

            steps = []
            for j in range(jmax):
                steps.append(mk_step(0, j))
                steps.append(mk_step(1, j))
            steps += [mk_tail(0), mk_tail(1)]
            return steps

        def oproj_units(g):
            """o_proj partial chunk: yT[:, tsl] = sum_h wo_h.T @ O_h, with
            psum evacuation rotated over DVE/ACT/Pool and a DMA per 4 tiles."""
            tsl = np.s_[512 * g:512 * (g + 1)]
            ybuf = yo.tile([128, NH_T, 512], BF16, tag="yo", name=f"yb{g}")

            # last chunk: finer DMA pieces so the final write drains with
            # the evacuations instead of after them
            per = 2 if g == NG - 1 else 4

            def mk(i):
                def emit():
                    py = spp.tile([128, 512], F32, tag="sp", name=f"y{g}_{i}")
                    # DoubleRow contracts both heads at once (kt dim = head);
                    # hi+lo residual terms accumulate in one group
                    wsl = np.s_[:, :, 128 * i:128 * (i + 1)]
                    for t, (oa, wb) in enumerate(((Oh_sb, woh_sb),
                                                  (Oh_sb, wol_sb),
                                                  (Ol_sb, woh_sb))):
                        nc.tensor.matmul(py[:], wb[wsl], oa[:, :, tsl],
                                         start=(t == 0), stop=(t == 2),
                                         perf_mode=DR)
                    # psum evacuation is DVE/ACT-only (GPSIMD cannot access
                    # PSUM); the 1/512 fp8 scale folds into the evac copy
                    m = KNOBS["ybuf_dve_mod"]
                    dve = (i % m == 2) if m else (i % 2 == 1)
                    if dve:
                        nc.vector.tensor_scalar_mul(ybuf[:, i, :], py[:],
                                                    1.0 / 512)
                    else:
                        nc.scalar.activation(ybuf[:, i, :], py[:],
                                             mybir.ActivationFunctionType.Copy,
                                             scale=1.0 / 512)
                    if i % per == per - 1:
                        lo = i - per + 1
                        nc.sync.dma_start(
                            out=d_yT[128 * lo:128 * (i + 1),
                                     tsl].rearrange("(a p) c -> p a c", p=128),
                            in_=ybuf[:, lo:i + 1, :])
                return emit

            return [mk(i) for i in range(NH_T)]

        def interleave(steps, fillers, lead=0):
            """Emit `lead` fillers up front (PE is in-order: a stalled step
            blocks everything emitted after it, so cover known step-0 latency
            with work emitted before it), then round-robin at ~1.5x rate so
            filler work front-loads and drains before the phase tail."""
            done = 0
            while done < min(lead, len(fillers)):
                fillers[done]()
                done += 1
            for si, s in enumerate(steps):
                s()
                want = max(done, (si + 1) * len(fillers) // len(steps))
                while done < min(want, len(fillers)):
                    fillers[done]()
                    done += 1
            while done < len(fillers):
                fillers[done]()
                done += 1

        # ================= schedule =================
        for u in proj_units_startup(0):
            u()
        for g in range(NG):
            # Filler assembly. Leads (emitted before attention step 0):
            # one ready-at-entry oproj unit + the next chunk's k projection
            # and k rope — the k rope chain gates the NEXT phase's first
            # scores, so it must start as early as possible. The xt
            # prefetch for g+2 is a filler placed after every rope swap
            # DMA of g+1 (the serialized DMA device is FIFO; 5.8us of xt
            # traffic in front of a swap stalls the next phase).
            ou = oproj_units(g - 1) if g > 0 else []
            pu = proj_units(g + 1) if g + 1 < NG else []
            head, tail = ou[:KNOBS["ou_head"]], ou[KNOBS["ou_head"]:]
            mixed = []
            for i in range(max(len(head), len(pu))):
                if KNOBS["ou_first"] and i < len(head):
                    mixed.append(head[i])
                if i < len(pu):
                    mixed.append(pu[i])
                if not KNOBS["ou_first"] and i < len(head):
                    mixed.append(head[i])
            fillers = mixed + tail
            interleave(attn_steps(g), fillers, lead=KNOBS["lead"])
            if g + 2 < NG:
                load_xt(g + 2)
        for u in oproj_units(NG - 1):
            u()

        if dbg:
            nc.sync.dma_start(out=d_qkv, in_=qkv_sb[:])
            nc.sync.dma_start(out=d_V, in_=V_sb[:])
            nc.sync.dma_start(out=d_O, in_=Oh_sb[:])

    nc.compile()
    return nc


_NC_CACHE = None


def _get_nc():
    global _NC_CACHE
    if _NC_CACHE is None:
        _NC_CACHE = _build()
    return _NC_CACHE


def _host_prep(positions, hidden_states, w_qkv, w_o):
    positions = np.asarray(positions, dtype=np.int32)
    hidden_states = np.asarray(hidden_states, dtype=np.float32)
    w_qkv = np.asarray(w_qkv, dtype=np.float32)
    w_o = np.asarray(w_o, dtype=np.float32)

    bf = ml_dtypes.bfloat16
    f8 = ml_dtypes.float8_e4m3

    # fp8 hi+lo residual split. Pre-scale by exact powers of two so both the
    # hi values and the ~3%-magnitude residuals sit in e4m3's normal range
    # (min normal 2^-6; unscaled w ~N(0,0.02) would be subnormal and the
    # residual would flush to zero). The x*8 * w*64 = 512x psum scale is
    # folded back out through the host rope tables (q/k path) and the wo
    # scaling (V path — softmax normalization is scale-invariant in V).
    SX, SW = 8.0, 64.0
    SINV = 1.0 / (SX * SW)

    def hilo(a, s):
        a = a * np.float32(s)
        hi = a.astype(f8)
        lo = (a - hi.astype(np.float32)).astype(f8)
        return np.ascontiguousarray(hi), np.ascontiguousarray(lo)

    xh, xl = hilo(np.ascontiguousarray(hidden_states.T), SX)
    # rope tables, host-computed: partition p holds rotation pair p%64 with
    # positions from ROW_MAP's t/h/w row. cdup = cos both halves;
    # sflip = [-sin; +sin] (so x*cdup + swap(x)*sflip rotates in place).
    pos_sel = positions[np.concatenate([ROW_MAP, ROW_MAP])].astype(np.float64)
    ang = pos_sel * np.concatenate([INVF, INVF]).astype(np.float64)[:, None]
    cdup = np.cos(ang) * SINV
    sflip = np.concatenate([-np.sin(ang[:64]), np.sin(ang[64:])], axis=0) * SINV
    # additive causal mask factors for the 128x128 diagonal boundary
    # subtile: invalid(dk, dq) = [dq < dk] = sum_p L[p,dk] * R[p,dq],
    #   L[p, dk] = [p <= dk],  R[p, dq] = -1e9 * [p == dq + 1]
    mask_l = (np.arange(128)[:, None] <= np.arange(128)[None, :]).astype(np.float32)
    mask_rb = np.zeros((128, 128), dtype=np.float32)
    mask_rb[np.arange(1, 128), np.arange(127)] = -1e9
    dq = np.arange(128)[None, :]
    dk = np.arange(128)[:, None]
    assert np.array_equal(mask_l.T @ mask_rb, np.where(dq < dk, -1e9, 0.0))
    # x64 folds the fp8 V(x512)/O(x8) scaling into rinv = 1/(64*r)
    ones = np.full((128, 128), 64.0, dtype=np.float32)

    q_size = N_HEADS * HD
    kv_size = N_KV * HD
    in_maps = []
    for c in range(NCORES):
        cols = [w_qkv[:, 2 * c * HD + PERM], w_qkv[:, (2 * c + 1) * HD + PERM]]
        kc = c // 2
        cols.append(w_qkv[:, q_size + kc * HD + PERM])
        cols.append(w_qkv[:, q_size + kv_size + kc * HD:q_size + kv_size + (kc + 1) * HD])
        wh, wl = hilo(np.concatenate(cols, axis=1), SW)
        woh, wol = hilo(w_o[2 * c * HD:(2 * c + 2) * HD], SW)
        in_maps.append({
            "xh": xh, "xl": xl, "wh": wh, "wl": wl, "woh": woh, "wol": wol,
            "cdup": np.ascontiguousarray(cdup).astype(bf),
            "sflip": np.ascontiguousarray(sflip).astype(bf),
            "mask_l": mask_l.astype(bf), "mask_rb": mask_rb.astype(bf),
            "ones": ones.astype(bf),
        })
    return in_maps


def kernel(positions, hidden_states, w_qkv, w_o):
    nc = _get_nc()
    in_maps = _host_prep(positions, hidden_states, w_qkv, w_o)
    # one retry: transient NRT/device errors (e.g. NRT_EXEC_UNIT_UNRECOVERABLE
    # from a wedged core) were observed to succeed on re-dispatch
    try:
        res = run_bass_kernel_spmd(nc, in_maps, core_ids=list(range(NCORES)))
    except Exception:
        import time
        time.sleep(2.0)
        res = run_bass_kernel_spmd(nc, in_maps, core_ids=list(range(NCORES)))
    yT = np.zeros((HIDDEN, T), dtype=np.float64)
    for c in range(NCORES):
        yT += np.asarray(res.results[c]["yT"], dtype=np.float64)
    return np.ascontiguousarray(yT.T).astype(np.float32)


# revision 92
# speedup vs baseline: 1.0250x; 1.0250x over previous
"""Trainium2 Bass kernel for Ernie4.5-VL attention (mRoPE + GQA causal attention).

Sharding: tensor-parallel over heads across 8 cores. Each core computes
2 q heads + its kv head (replicated per core pair): qkv projection
(q/k feature-major, V token-major directly — no transposes), interleaved
mRoPE (via a host-side even/odd column permutation of the q/k weight
slices so the rotation becomes two contiguous partition halves), causal
attention with unnormalized softmax (denominator via bf16 tile adds +
one all-ones matmul), and the o_proj partial product. Host sums the 8
partial outputs.

All tensors move through SBUF/DRAM as bf16; matmuls are bf16 in / fp32
psum out; psum evacuations round once to bf16. Schedule: a flat
software pipeline where attention chunk g's score->exp->AV steps are
interleaved (emission-order round-robin) with chunk g+1's projection
matmuls and chunk g-1's o_proj — the PE fills exp (ACT) latency with
projection work instead of stalling, which also keeps the PE p-state
ramped.
"""
import numpy as np
import ml_dtypes
from contextlib import ExitStack

import concourse.bacc as bacc
import concourse.tile as tile
from concourse import mybir
from concourse.bass_utils import run_bass_kernel_spmd

HIDDEN = 2048
T = 2048
N_HEADS = 16
N_KV = 4
HD = 128
THETA = 500000.0
NCORES = 8
SCALE = HD ** -0.5

F32 = mybir.dt.float32
BF16 = mybir.dt.bfloat16
FP8 = mybir.dt.float8e4
I32 = mybir.dt.int32
DR = mybir.MatmulPerfMode.DoubleRow

# within-head column permutation: evens then odds (so interleaved rope pairs
# become two contiguous partition halves in feature-major layout)
PERM = np.concatenate([np.arange(0, HD, 2), np.arange(1, HD, 2)])
# pair index p (0..63): p<44: even->pos row 1 (h), odd->row 2 (w); p>=44: row 0 (t)
ROW_MAP = np.array([(1 if p % 2 == 0 else 2) if p < 44 else 0 for p in range(64)])
INVF = (THETA ** (-(np.arange(64, dtype=np.float64) / 64))).astype(np.float32)

NT = T // 128      # 16 token tiles
NG = T // 512      # 4 token chunks
NH_T = HIDDEN // 128  # 16 hidden tiles

# schedule knobs (tuned via TimelineSim sweep)
KNOBS = {
    "ou_first": False,   # oproj units before proj units in the filler mix
    "rope_add_dve": False,  # rope final add on DVE (False: Pool)
    "ybuf_dve_mod": 0,   # ybuf evac: i % mod == 2 -> DVE, else ACT (0: 50/50)
    "lead": 2,           # fillers emitted before step 0 of each phase
    "xt_at_end": True,   # prefetch xt(g+2) at phase end (False: start)
    "ou_head": 8,        # oproj units mixed in early (rest appended at end)
}


def _build(dbg=False):
    nc = bacc.Bacc("TRN2", target_bir_lowering=False, debug=False)
    d_xh = nc.dram_tensor("xh", [HIDDEN, T], FP8, kind="ExternalInput").ap()
    d_xl = nc.dram_tensor("xl", [HIDDEN, T], FP8, kind="ExternalInput").ap()
    d_wh = nc.dram_tensor("wh", [HIDDEN, 512], FP8, kind="ExternalInput").ap()
    d_wl = nc.dram_tensor("wl", [HIDDEN, 512], FP8, kind="ExternalInput").ap()
    d_wo = nc.dram_tensor("wo_slice", [256, HIDDEN], BF16, kind="ExternalInput").ap()
    d_cdup = nc.dram_tensor("cdup", [128, T], BF16, kind="ExternalInput").ap()
    d_sflip = nc.dram_tensor("sflip", [128, T], BF16, kind="ExternalInput").ap()
    d_mL = nc.dram_tensor("mask_l", [128, 128], BF16, kind="ExternalInput").ap()
    d_mR = nc.dram_tensor("mask_rb", [128, 128], BF16, kind="ExternalInput").ap()
    d_ones = nc.dram_tensor("ones", [128, 128], BF16, kind="ExternalInput").ap()
    d_yT = nc.dram_tensor("yT", [HIDDEN, T], BF16, kind="ExternalOutput").ap()
    if dbg:
        d_qkv = nc.dram_tensor("dbg_qkv", [128, 3, T], BF16, kind="ExternalOutput").ap()
        d_V = nc.dram_tensor("dbg_V", [128, NT, 128], BF16, kind="ExternalOutput").ap()
        d_O = nc.dram_tensor("dbg_O", [128, 2, T], BF16, kind="ExternalOutput").ap()

    with tile.TileContext(nc) as tc, ExitStack() as ctx:
        const = ctx.enter_context(tc.tile_pool(name="const", bufs=1))
        big = ctx.enter_context(tc.tile_pool(name="big", bufs=1))

        # resident tiles. qkv projection runs as fp8e4m3 hi+lo residual
        # DoubleRow matmuls (xh@wh + xh@wl + xl@wh = 1.5 half-rate passes,
        # 0.75x the bf16 cost at ~bf16 accuracy).
        wh_sb = const.tile([128, NH_T, 512], FP8)
        wl_sb = const.tile([128, NH_T, 512], FP8)
        wo_sb = const.tile([128, 2, HIDDEN], BF16)      # o_proj rows
        mL_sb = const.tile([128, 128], BF16)            # causal mask, left factor
        mR_sb = const.tile([128, 128], BF16)            # causal mask, right factor
        ones_sb = const.tile([128, 128], BF16)
        qkv_sb = big.tile([128, 3, T], BF16)            # q0|q1|k feature-major (roped)
        V_sb = big.tile([128, NT, 128], BF16)           # V token-major
        O_sb = big.tile([128, 2, T], BF16)              # attention out, feature-major
        cdup = big.tile([128, T], BF16)                 # cos table (dup halves)
        sflip = big.tile([128, T], BF16)                # sin table ([-s; s])

        # PSUM budget (8 banks): projection feature accum 2 (k/q0/q1 rotate —
        # a feature's bank frees once its rope has read it) + V-direct 1 +
        # shared(scores/o_proj) 3 + AV accum 2 (both heads in flight).
        xtp = ctx.enter_context(tc.tile_pool(name="xt", bufs=2))
        qkvp = ctx.enter_context(tc.tile_pool(name="qkvp", bufs=2, space="PSUM"))
        vdp = ctx.enter_context(tc.tile_pool(name="vdp", bufs=1, space="PSUM"))
        spp = ctx.enter_context(tc.tile_pool(name="spp", bufs=3, space="PSUM"))
        avp = ctx.enter_context(tc.tile_pool(name="avp", bufs=2, space="PSUM"))
        rp = ctx.enter_context(tc.tile_pool(name="rope", bufs=2))
        ep = ctx.enter_context(tc.tile_pool(name="ep", bufs=8))
        rv = ctx.enter_context(tc.tile_pool(name="rv", bufs=2))
        racc = ctx.enter_context(tc.tile_pool(name="racc", bufs=2))
        yo = ctx.enter_context(tc.tile_pool(name="yo", bufs=2))

        # ---- startup DMAs, ordered so the first projection matmuls unblock
        # as early as possible (cos/sin rope tables are host-computed)
        xt_tiles = {}

        def load_xt(g):
            th = xtp.tile([128, NH_T, 512], FP8, tag="xth", name=f"xth_{g}")
            tl = xtp.tile([128, NH_T, 512], FP8, tag="xtl", name=f"xtl_{g}")
            xt_tiles[g] = (th, tl)
            for q4 in range(4):
                sl = np.s_[:, 4 * q4:4 * (q4 + 1), :]
                dsl = np.s_[512 * q4:512 * (q4 + 1), 512 * g:512 * (g + 1)]
                nc.sync.dma_start(
                    out=th[sl], in_=d_xh[dsl].rearrange("(a p) c -> p a c", p=128))
                nc.sync.dma_start(
                    out=tl[sl], in_=d_xl[dsl].rearrange("(a p) c -> p a c", p=128))

        xt0h = xtp.tile([128, NH_T, 512], FP8, tag="xth", name="xth_0")
        xt0l = xtp.tile([128, NH_T, 512], FP8, tag="xtl", name="xtl_0")
        xt_tiles[0] = (xt0h, xt0l)
        for lo, hi in ((0, 2), (2, 4), (4, 8), (8, 16)):
            for w_d, w_s, x_d, x_s in ((wh_sb, d_wh, xt0h, d_xh),
                                       (wl_sb, d_wl, xt0l, d_xl)):
                nc.sync.dma_start(
                    out=w_d[:, lo:hi, :],
                    in_=w_s[128 * lo:128 * hi, :].rearrange(
                        "(a p) c -> p a c", p=128))
                nc.sync.dma_start(
                    out=x_d[:, lo:hi, :],
                    in_=x_s[128 * lo:128 * hi, 0:512].rearrange(
                        "(a p) c -> p a c", p=128))
        nc.sync.dma_start(out=cdup, in_=d_cdup)
        nc.sync.dma_start(out=sflip, in_=d_sflip)
        nc.sync.dma_start(out=mL_sb, in_=d_mL)
        nc.sync.dma_start(out=mR_sb, in_=d_mR)
        nc.sync.dma_start(out=ones_sb, in_=d_ones)

        def load_wo():
            # deferred until after rope(0)'s swap DMAs: the FIFO DMA device
            # must not put bulk traffic in front of the latency-critical
            # swaps (wo isn't needed until oproj(0) in phase 1)
            for q2 in range(2):
                nc.sync.dma_start(
                    out=wo_sb[:, :, 1024 * q2:1024 * (q2 + 1)],
                    in_=d_wo[:, 1024 * q2:1024 * (q2 + 1)].rearrange(
                        "(a p) c -> p a c", p=128))

        # ================= stage emitters =================
        proj_state = {}

        def proj_alloc(g):
            assert g in xt_tiles, f"xt chunk {g} not prefetched"
            proj_state[g] = {
                "ps": {},
                "vd": vdp.tile([128, 4, 128], F32, tag="vd", name=f"vd_{g}"),
            }

        def proj_ps(g, i):
            # lazy per-feature psum: the 2-deep ring reuses a feature's bank
            # only after its rope has consumed it (k -> q0 -> q1 order)
            st = proj_state[g]
            if i not in st["ps"]:
                st["ps"][i] = qkvp.tile([128, 512], F32, tag="qkvps",
                                        name=f"qkvps_{g}_{i}")
            return st["ps"][i]

        NP_ = NH_T // 2   # DoubleRow passes (2 hidden tiles per pass)

        def mk_proj_feat(g, i, us):
            """One feature's (q0/q1/k) psum accumulation over DoubleRow
            passes `us`, three residual terms per pass."""
            def emit():
                if i == 2 and us[0] == 0:
                    proj_alloc(g)
                ps = proj_ps(g, i)
                xh_t, xl_t = xt_tiles[g]
                for u in us:
                    hsl = np.s_[:, 2 * u:2 * (u + 1)]
                    for t, (xa, wb) in enumerate(((xh_t, wh_sb), (xh_t, wl_sb),
                                                  (xl_t, wh_sb))):
                        nc.tensor.matmul(
                            ps[:],
                            wb[:, 2 * u:2 * (u + 1), 128 * i:128 * (i + 1)],
                            xa[hsl],
                            start=(u == 0 and t == 0),
                            stop=(u == NP_ - 1 and t == 2), perf_mode=DR)
            return emit

        def mk_proj_v(g, tt):
            # one token tile's full contraction as a contiguous start..stop
            # group: interleaved accumulation groups at different offsets of
            # the SAME psum bank produce wrong results on hardware
            def emit():
                st = proj_state[g]
                xh_t, xl_t = xt_tiles[g]
                for u in range(NP_):
                    for t, (xa, wb) in enumerate(((xh_t, wh_sb), (xh_t, wl_sb),
                                                  (xl_t, wh_sb))):
                        nc.tensor.matmul(
                            st["vd"][:, tt, :],
                            xa[:, 2 * u:2 * (u + 1), 128 * tt:128 * (tt + 1)],
                            wb[:, 2 * u:2 * (u + 1), 384:512],
                            start=(u == 0 and t == 0),
                            stop=(u == NP_ - 1 and t == 2), perf_mode=DR)
            return emit

        def proj_units_startup(g):
            """Chunk 0: k and q0 pass-major (follows the startup DMA arrival
            order, using both psum ring slots), then their ropes free k's
            bank for q1."""
            units = []

            def mk(u):
                def emit():
                    if u == 0:
                        proj_alloc(g)
                    xh_t, xl_t = xt_tiles[g]
                    for i in (2, 0):
                        ps = proj_ps(g, i)
                        for t, (xa, wb) in enumerate(((xh_t, wh_sb),
                                                      (xh_t, wl_sb),
                                                      (xl_t, wh_sb))):
                            nc.tensor.matmul(
                                ps[:],
                                wb[:, 2 * u:2 * (u + 1), 128 * i:128 * (i + 1)],
                                xa[:, 2 * u:2 * (u + 1), :],
                                start=(u == 0 and t == 0),
                                stop=(u == NP_ - 1 and t == 2), perf_mode=DR)
                return emit

            def late_loads():
                load_xt(1)
                load_wo()

            ru = rope_units(g, add_dve=True)   # DVE is idle at startup
            duos = [list(range(2 * q, 2 * q + 2)) for q in range(4)]
            units = [mk(u) for u in range(NP_)] + [ru[0], ru[2], late_loads]
            units += [mk_proj_v(g, tt) for tt in range(4)] + [ru[1]]
            units += [mk_proj_feat(g, 1, u2) for u2 in duos] + [ru[3]]
            return units

        def proj_units(g):
            """Chunk g's projection + rope as filler units, ordered so each
            feature's psum completes early and its rope follows immediately:
            k first (gates next phase's scores), then q0, V, q1."""
            ru = rope_units(g)
            duos = [list(range(2 * q, 2 * q + 2)) for q in range(4)]
            units = [mk_proj_feat(g, 2, u2) for u2 in duos] + [ru[0]]
            units += [mk_proj_feat(g, 0, u2) for u2 in duos] + [ru[2]]
            units += [mk_proj_v(g, tt) for tt in range(4)] + [ru[1]]
            units += [mk_proj_feat(g, 1, u2) for u2 in duos] + [ru[3]]
            return units

        def rope_units(g, add_dve=None):
            """V evac (Pool) + mRoPE for k/q0/q1 of chunk g as filler units,
            reading the projection psums (swapped halves via a bf16 scratch
            + SBUF->SBUF DMA), writing bf16 qkv_sb once. k first: it gates
            chunk g's scores."""
            tsl = np.s_[512 * g:512 * (g + 1)]
            xs = rp.tile([128, 3, 512], BF16, tag="xs", name=f"xs{g}")

            def mk_rope(t3):
                def emit():
                    st = proj_state[g]
                    psx = st["ps"][t3]
                    x = qkv_sb[:, t3, tsl]
                    xraw = rp.tile([128, 512], BF16, tag="xraw",
                                   name=f"xr_{g}_{t3}")
                    nc.vector.tensor_copy(xraw[:], psx[:])
                    nc.sync.dma_start(out=xs[0:64, t3, :], in_=xraw[64:128, :])
                    nc.sync.dma_start(out=xs[64:128, t3, :], in_=xraw[0:64, :])
                    t1 = rp.tile([128, 512], F32, tag="t1", name=f"t1_{g}_{t3}")
                    t2 = rp.tile([128, 512], F32, tag="t2", name=f"t2_{g}_{t3}")
                    # GPSIMD cannot touch PSUM: t1 (psum read) stays on DVE;
                    # t2 is SBUF-only so Pool takes it; the add returns to
                    # DVE (Pool's ~1.2us ops would serialize the k chain
                    # that gates the next phase's first scores)
                    nc.vector.tensor_mul(t1[:], psx[:], cdup[:, tsl])
                    nc.gpsimd.tensor_mul(t2[:], xs[:, t3, :], sflip[:, tsl])
                    on_dve = KNOBS["rope_add_dve"] if add_dve is None else add_dve
                    if on_dve:
                        nc.vector.tensor_add(x, t1[:], t2[:])
                    else:
                        nc.gpsimd.tensor_add(x, t1[:], t2[:])
                return emit

            def mk_vevac():
                def emit():
                    st = proj_state[g]
                    for tt in range(4):
                        nc.vector.tensor_copy(V_sb[:, 4 * g + tt, :],
                                              st["vd"][:, tt, :])
                return emit

            return [mk_rope(2), mk_vevac(), mk_rope(0), mk_rope(1)]

        def attn_steps(g):
            """Flat list of per-j-step emitters for both heads of chunk g.
            Scores run one step ahead of AV; head-0's denominator tail is
            emitted two steps into head 1 so its latency hides behind
            head-1 scores."""
            tsl = np.s_[512 * g:512 * (g + 1)]
            jmax = 4 * g + 4
            state = {}

            def head_alloc(h):
                state[h] = {
                    "po": avp.tile([128, 512], F32, tag="av", name=f"po{g}_{h}"),
                    "ra": racc.tile([128, 512], BF16, tag="ra", name=f"ra{g}_{h}"),
                    "Es": [None] * jmax,
                }

            # diagonal tiles (m = j-4g >= 0): queries [0:128m) can never see
            # this key tile, so scores/exp/AV/row-sum all trim to [128m:512]
            # and only the 128-wide boundary subtile needs the factored
            # triangular mask. The psum/E region left of 128m stays garbage
            # and is never read.
            def lo_of(j):
                return max(0, 128 * (j - 4 * g))

            def mk_av(h, j, stop):
                lo = lo_of(j)
                st = state[h]
                nc.tensor.matmul(st["po"][:, lo:], V_sb[:, j, :],
                                 st["Es"][j][:, lo:],
                                 start=(j == 0), stop=stop,
                                 skip_group_check=True)

            def mk_step(h, j):
                def emit():
                    if j == 0:
                        head_alloc(h)
                    st = state[h]
                    qc = qkv_sb[:, h, tsl]
                    m = j - 4 * g
                    lo = lo_of(j)
                    ps = spp.tile([128, 512], F32, tag="sp", name=f"s{g}_{h}_{j}")
                    nc.tensor.matmul(ps[:, lo:], qkv_sb[:, 2, 128 * j:128 * (j + 1)],
                                     qc[:, lo:], start=True, stop=(m < 0),
                                     skip_group_check=True)
                    if m >= 0:
                        nc.tensor.matmul(ps[:, lo:lo + 128], mL_sb[:], mR_sb[:],
                                         start=False, stop=True,
                                         skip_group_check=True)
                    E = ep.tile([128, 512], BF16, tag="e", name=f"e{g}_{h}_{j}")
                    st["Es"][j] = E
                    nc.scalar.activation(E[:, lo:], ps[:, lo:],
                                         mybir.ActivationFunctionType.Exp,
                                         scale=SCALE)
                    # row-sum partial accumulation on DVE (in-order engine:
                    # a single chain has the same throughput as two and a
                    # shorter tail)
                    if j == 0:
                        nc.vector.tensor_copy(st["ra"][:], E[:])
                    else:
                        nc.vector.tensor_add(st["ra"][:, lo:], st["ra"][:, lo:],
                                             E[:, lo:])
                    if j >= 1:
                        mk_av(h, j - 1, stop=False)
                return emit

            def mk_tail(h):
                def emit():
                    st = state[h]
                    mk_av(h, jmax - 1, stop=True)
                    # r broadcast across partitions via one all-ones matmul
                    pr = spp.tile([128, 512], F32, tag="sp", name=f"pr{g}_{h}")
                    nc.tensor.matmul(pr[:], ones_sb[:], st["ra"][:],
                                     start=True, stop=True)
                    rinv = rv.tile([128, 512], F32, tag="rv", name=f"rinv{g}_{h}")
                    nc.vector.reciprocal(rinv[:], pr[:])
                    nc.vector.tensor_mul(O_sb[:, h, tsl], st["po"][:], rinv[:])
                return emit

            steps = []
            for j in range(jmax):
                steps.append(mk_step(0, j))
                steps.append(mk_step(1, j))
            steps += [mk_tail(0), mk_tail(1)]
            return steps

        def oproj_units(g):
            """o_proj partial chunk: yT[:, tsl] = sum_h wo_h.T @ O_h, with
            psum evacuation rotated over DVE/ACT/Pool and a DMA per 4 tiles."""
            tsl = np.s_[512 * g:512 * (g + 1)]
            ybuf = yo.tile([128, NH_T, 512], BF16, tag="yo", name=f"yb{g}")

            # last chunk: finer DMA pieces so the final write drains with
            # the evacuations instead of after them
            per = 2 if g == NG - 1 else 4

            def mk(i):
                def emit():
                    py = spp.tile([128, 512], F32, tag="sp", name=f"y{g}_{i}")
                    for h in range(2):
                        nc.tensor.matmul(py[:], wo_sb[:, h, 128 * i:128 * (i + 1)],
                                         O_sb[:, h, tsl],
                                         start=(h == 0), stop=(h == 1))
                    # psum evacuation is DVE/ACT-only (GPSIMD cannot access
                    # PSUM); rotation split per KNOBS
                    m = KNOBS["ybuf_dve_mod"]
                    dve = (i % m == 2) if m else (i % 2 == 1)
                    if dve:
                        nc.vector.tensor_copy(ybuf[:, i, :], py[:])
                    else:
                        nc.scalar.copy(ybuf[:, i, :], py[:])
                    if i % per == per - 1:
                        lo = i - per + 1
                        nc.sync.dma_start(
                            out=d_yT[128 * lo:128 * (i + 1),
                                     tsl].rearrange("(a p) c -> p a c", p=128),
                            in_=ybuf[:, lo:i + 1, :])
                return emit

            return [mk(i) for i in range(NH_T)]

        def interleave(steps, fillers, lead=0):
            """Emit `lead` fillers up front (PE is in-order: a stalled step
            blocks everything emitted after it, so cover known step-0 latency
            with work emitted before it), then round-robin at ~1.5x rate so
            filler work front-loads and drains before the phase tail."""
            done = 0
            while done < min(lead, len(fillers)):
                fillers[done]()
                done += 1
            for si, s in enumerate(steps):
                s()
                want = max(done, (si + 1) * len(fillers) // len(steps))
                while done < min(want, len(fillers)):
                    fillers[done]()
                    done += 1
            while done < len(fillers):
                fillers[done]()
                done += 1

        # ================= schedule =================
        for u in proj_units_startup(0):
            u()
        for g in range(NG):
            # Filler assembly. Leads (emitted before attention step 0):
            # one ready-at-entry oproj unit + the next chunk's k projection
            # and k rope — the k rope chain gates the NEXT phase's first
            # scores, so it must start as early as possible. The xt
            # prefetch for g+2 is a filler placed after every rope swap
            # DMA of g+1 (the serialized DMA device is FIFO; 5.8us of xt
            # traffic in front of a swap stalls the next phase).
            ou = oproj_units(g - 1) if g > 0 else []
            pu = proj_units(g + 1) if g + 1 < NG else []
            head, tail = ou[:KNOBS["ou_head"]], ou[KNOBS["ou_head"]:]
            mixed = []
            for i in range(max(len(head), len(pu))):
                if KNOBS["ou_first"] and i < len(head):
                    mixed.append(head[i])
                if i < len(pu):
                    mixed.append(pu[i])
                if not KNOBS["ou_first"] and i < len(head):
                    mixed.append(head[i])
            fillers = mixed + tail
            interleave(attn_steps(g), fillers, lead=KNOBS["lead"])
            if g + 2 < NG:
                load_xt(g + 2)
        for u in oproj_units(NG - 1):
            u()

        if dbg:
            nc.sync.dma_start(out=d_qkv, in_=qkv_sb[:])
            nc.sync.dma_start(out=d_V, in_=V_sb[:])
            nc.sync.dma_start(out=d_O, in_=O_sb[:])

    nc.compile()
    return nc


_NC_CACHE = None


def _get_nc():
    global _NC_CACHE
    if _NC_CACHE is None:
        _NC_CACHE = _build()
    return _NC_CACHE


def _host_prep(positions, hidden_states, w_qkv, w_o):
    positions = np.asarray(positions, dtype=np.int32)
    hidden_states = np.asarray(hidden_states, dtype=np.float32)
    w_qkv = np.asarray(w_qkv, dtype=np.float32)
    w_o = np.asarray(w_o, dtype=np.float32)

    bf = ml_dtypes.bfloat16
    f8 = ml_dtypes.float8_e4m3

    # fp8 hi+lo residual split. Pre-scale by exact powers of two so both the
    # hi values and the ~3%-magnitude residuals sit in e4m3's normal range
    # (min normal 2^-6; unscaled w ~N(0,0.02) would be subnormal and the
    # residual would flush to zero). The x*8 * w*64 = 512x psum scale is
    # folded back out through the host rope tables (q/k path) and the wo
    # scaling (V path — softmax normalization is scale-invariant in V).
    SX, SW = 8.0, 64.0
    SINV = 1.0 / (SX * SW)

    def hilo(a, s):
        a = a * np.float32(s)
        hi = a.astype(f8)
        lo = (a - hi.astype(np.float32)).astype(f8)
        return np.ascontiguousarray(hi), np.ascontiguousarray(lo)

    xh, xl = hilo(np.ascontiguousarray(hidden_states.T), SX)
    # rope tables, host-computed: partition p holds rotation pair p%64 with
    # positions from ROW_MAP's t/h/w row. cdup = cos both halves;
    # sflip = [-sin; +sin] (so x*cdup + swap(x)*sflip rotates in place).
    pos_sel = positions[np.concatenate([ROW_MAP, ROW_MAP])].astype(np.float64)
    ang = pos_sel * np.concatenate([INVF, INVF]).astype(np.float64)[:, None]
    cdup = np.cos(ang) * SINV
    sflip = np.concatenate([-np.sin(ang[:64]), np.sin(ang[64:])], axis=0) * SINV
    # additive causal mask factors for the 128x128 diagonal boundary
    # subtile: invalid(dk, dq) = [dq < dk] = sum_p L[p,dk] * R[p,dq],
    #   L[p, dk] = [p <= dk],  R[p, dq] = -1e9 * [p == dq + 1]
    mask_l = (np.arange(128)[:, None] <= np.arange(128)[None, :]).astype(np.float32)
    mask_rb = np.zeros((128, 128), dtype=np.float32)
    mask_rb[np.arange(1, 128), np.arange(127)] = -1e9
    dq = np.arange(128)[None, :]
    dk = np.arange(128)[:, None]
    assert np.array_equal(mask_l.T @ mask_rb, np.where(dq < dk, -1e9, 0.0))
    ones = np.ones((128, 128), dtype=np.float32)

    q_size = N_HEADS * HD
    kv_size = N_KV * HD
    in_maps = []
    for c in range(NCORES):
        cols = [w_qkv[:, 2 * c * HD + PERM], w_qkv[:, (2 * c + 1) * HD + PERM]]
        kc = c // 2
        cols.append(w_qkv[:, q_size + kc * HD + PERM])
        cols.append(w_qkv[:, q_size + kv_size + kc * HD:q_size + kv_size + (kc + 1) * HD])
        wh, wl = hilo(np.concatenate(cols, axis=1), SW)
        wo_slice = np.ascontiguousarray(
            w_o[2 * c * HD:(2 * c + 2) * HD] * np.float32(SINV)).astype(bf)
        in_maps.append({
            "xh": xh, "xl": xl, "wh": wh, "wl": wl, "wo_slice": wo_slice,
            "cdup": np.ascontiguousarray(cdup).astype(bf),
            "sflip": np.ascontiguousarray(sflip).astype(bf),
            "mask_l": mask_l.astype(bf), "mask_rb": mask_rb.astype(bf),
            "ones": ones.astype(bf),
        })
    return in_maps


def kernel(positions, hidden_states, w_qkv, w_o):
    nc = _get_nc()
    in_maps = _host_prep(positions, hidden_states, w_qkv, w_o)
    # one retry: transient NRT/device errors (e.g. NRT_EXEC_UNIT_UNRECOVERABLE
    # from a wedged core) were observed to succeed on re-dispatch
    try:
        res = run_bass_kernel_spmd(nc, in_maps, core_ids=list(range(NCORES)))
    except Exception:
        import time
        time.sleep(2.0)
        res = run_bass_kernel_spmd(nc, in_maps, core_ids=list(range(NCORES)))
    yT = np.zeros((HIDDEN, T), dtype=np.float64)
    for c in range(NCORES):
        yT += np.asarray(res.results[c]["yT"], dtype=np.float64)
    return np.ascontiguousarray(yT.T).astype(np.float32)


# revision 93
# speedup vs baseline: 1.0293x; 1.0042x over previous
"""Trainium2 Bass kernel for Ernie4.5-VL attention (mRoPE + GQA causal attention).

Sharding: tensor-parallel over heads across 8 cores. Each core computes
2 q heads + its kv head (replicated per core pair): qkv projection
(q/k feature-major, V token-major directly — no transposes), interleaved
mRoPE (via a host-side even/odd column permutation of the q/k weight
slices so the rotation becomes two contiguous partition halves), causal
attention with unnormalized softmax (denominator via bf16 tile adds +
one all-ones matmul), and the o_proj partial product. Host sums the 8
partial outputs.

All tensors move through SBUF/DRAM as bf16; matmuls are bf16 in / fp32
psum out; psum evacuations round once to bf16. Schedule: a flat
software pipeline where attention chunk g's score->exp->AV steps are
interleaved (emission-order round-robin) with chunk g+1's projection
matmuls and chunk g-1's o_proj — the PE fills exp (ACT) latency with
projection work instead of stalling, which also keeps the PE p-state
ramped.
"""
import numpy as np
import ml_dtypes
from contextlib import ExitStack

import concourse.bacc as bacc
import concourse.tile as tile
from concourse import mybir
from concourse.bass_utils import run_bass_kernel_spmd

HIDDEN = 2048
T = 2048
N_HEADS = 16
N_KV = 4
HD = 128
THETA = 500000.0
NCORES = 8
SCALE = HD ** -0.5

F32 = mybir.dt.float32
BF16 = mybir.dt.bfloat16
FP8 = mybir.dt.float8e4
I32 = mybir.dt.int32
DR = mybir.MatmulPerfMode.DoubleRow

# within-head column permutation: evens then odds (so interleaved rope pairs
# become two contiguous partition halves in feature-major layout)
PERM = np.concatenate([np.arange(0, HD, 2), np.arange(1, HD, 2)])
# pair index p (0..63): p<44: even->pos row 1 (h), odd->row 2 (w); p>=44: row 0 (t)
ROW_MAP = np.array([(1 if p % 2 == 0 else 2) if p < 44 else 0 for p in range(64)])
INVF = (THETA ** (-(np.arange(64, dtype=np.float64) / 64))).astype(np.float32)

NT = T // 128      # 16 token tiles
NG = T // 512      # 4 token chunks
NH_T = HIDDEN // 128  # 16 hidden tiles

# schedule knobs (tuned via TimelineSim sweep)
KNOBS = {
    "ou_first": True,    # oproj units before proj units in the filler mix
    "rope_add_dve": False,  # rope final add on DVE (False: Pool)
    "ybuf_dve_mod": 0,   # ybuf evac: i % mod == 2 -> DVE, else ACT (0: 50/50)
    "lead": 1,           # fillers emitted before step 0 of each phase
    "xt_at_end": True,   # prefetch xt(g+2) at phase end (False: start)
    "ou_head": 8,        # oproj units mixed in early (rest appended at end)
}


def _build(dbg=False):
    nc = bacc.Bacc("TRN2", target_bir_lowering=False, debug=False)
    d_xh = nc.dram_tensor("xh", [HIDDEN, T], FP8, kind="ExternalInput").ap()
    d_xl = nc.dram_tensor("xl", [HIDDEN, T], FP8, kind="ExternalInput").ap()
    d_wh = nc.dram_tensor("wh", [HIDDEN, 512], FP8, kind="ExternalInput").ap()
    d_wl = nc.dram_tensor("wl", [HIDDEN, 512], FP8, kind="ExternalInput").ap()
    d_wo = nc.dram_tensor("wo_slice", [256, HIDDEN], BF16, kind="ExternalInput").ap()
    d_cdup = nc.dram_tensor("cdup", [128, T], BF16, kind="ExternalInput").ap()
    d_sflip = nc.dram_tensor("sflip", [128, T], BF16, kind="ExternalInput").ap()
    d_mL = nc.dram_tensor("mask_l", [128, 128], BF16, kind="ExternalInput").ap()
    d_mR = nc.dram_tensor("mask_rb", [128, 128], BF16, kind="ExternalInput").ap()
    d_ones = nc.dram_tensor("ones", [128, 128], BF16, kind="ExternalInput").ap()
    d_yT = nc.dram_tensor("yT", [HIDDEN, T], BF16, kind="ExternalOutput").ap()
    if dbg:
        d_qkv = nc.dram_tensor("dbg_qkv", [128, 3, T], BF16, kind="ExternalOutput").ap()
        d_V = nc.dram_tensor("dbg_V", [128, NT, 128], BF16, kind="ExternalOutput").ap()
        d_O = nc.dram_tensor("dbg_O", [128, 2, T], BF16, kind="ExternalOutput").ap()

    with tile.TileContext(nc) as tc, ExitStack() as ctx:
        const = ctx.enter_context(tc.tile_pool(name="const", bufs=1))
        big = ctx.enter_context(tc.tile_pool(name="big", bufs=1))

        # resident tiles. qkv projection runs as fp8e4m3 hi+lo residual
        # DoubleRow matmuls (xh@wh + xh@wl + xl@wh = 1.5 half-rate passes,
        # 0.75x the bf16 cost at ~bf16 accuracy).
        wh_sb = const.tile([128, NH_T, 512], FP8)
        wl_sb = const.tile([128, NH_T, 512], FP8)
        wo_sb = const.tile([128, 2, HIDDEN], BF16)      # o_proj rows
        mL_sb = const.tile([128, 128], BF16)            # causal mask, left factor
        mR_sb = const.tile([128, 128], BF16)            # causal mask, right factor
        ones_sb = const.tile([128, 128], BF16)
        qkv_sb = big.tile([128, 3, T], BF16)            # q0|q1|k feature-major (roped)
        V_sb = big.tile([128, NT, 128], BF16)           # V token-major
        O_sb = big.tile([128, 2, T], BF16)              # attention out, feature-major
        cdup = big.tile([128, T], BF16)                 # cos table (dup halves)
        sflip = big.tile([128, T], BF16)                # sin table ([-s; s])

        # PSUM budget (8 banks): projection feature accum 2 (k/q0/q1 rotate —
        # a feature's bank frees once its rope has read it) + V-direct 1 +
        # shared(scores/o_proj) 3 + AV accum 2 (both heads in flight).
        xtp = ctx.enter_context(tc.tile_pool(name="xt", bufs=2))
        qkvp = ctx.enter_context(tc.tile_pool(name="qkvp", bufs=2, space="PSUM"))
        vdp = ctx.enter_context(tc.tile_pool(name="vdp", bufs=1, space="PSUM"))
        spp = ctx.enter_context(tc.tile_pool(name="spp", bufs=3, space="PSUM"))
        avp = ctx.enter_context(tc.tile_pool(name="avp", bufs=2, space="PSUM"))
        rp = ctx.enter_context(tc.tile_pool(name="rope", bufs=2))
        ep = ctx.enter_context(tc.tile_pool(name="ep", bufs=8))
        rv = ctx.enter_context(tc.tile_pool(name="rv", bufs=2))
        racc = ctx.enter_context(tc.tile_pool(name="racc", bufs=2))
        yo = ctx.enter_context(tc.tile_pool(name="yo", bufs=2))

        # ---- startup DMAs, ordered so the first projection matmuls unblock
        # as early as possible (cos/sin rope tables are host-computed)
        xt_tiles = {}

        def load_xt(g):
            th = xtp.tile([128, NH_T, 512], FP8, tag="xth", name=f"xth_{g}")
            tl = xtp.tile([128, NH_T, 512], FP8, tag="xtl", name=f"xtl_{g}")
            xt_tiles[g] = (th, tl)
            for q4 in range(4):
                sl = np.s_[:, 4 * q4:4 * (q4 + 1), :]
                dsl = np.s_[512 * q4:512 * (q4 + 1), 512 * g:512 * (g + 1)]
                nc.sync.dma_start(
                    out=th[sl], in_=d_xh[dsl].rearrange("(a p) c -> p a c", p=128))
                nc.sync.dma_start(
                    out=tl[sl], in_=d_xl[dsl].rearrange("(a p) c -> p a c", p=128))

        xt0h = xtp.tile([128, NH_T, 512], FP8, tag="xth", name="xth_0")
        xt0l = xtp.tile([128, NH_T, 512], FP8, tag="xtl", name="xtl_0")
        xt_tiles[0] = (xt0h, xt0l)
        for lo, hi in ((0, 2), (2, 4), (4, 8), (8, 16)):
            for w_d, w_s, x_d, x_s in ((wh_sb, d_wh, xt0h, d_xh),
                                       (wl_sb, d_wl, xt0l, d_xl)):
                nc.sync.dma_start(
                    out=w_d[:, lo:hi, :],
                    in_=w_s[128 * lo:128 * hi, :].rearrange(
                        "(a p) c -> p a c", p=128))
                nc.sync.dma_start(
                    out=x_d[:, lo:hi, :],
                    in_=x_s[128 * lo:128 * hi, 0:512].rearrange(
                        "(a p) c -> p a c", p=128))
        nc.sync.dma_start(out=cdup, in_=d_cdup)
        nc.sync.dma_start(out=sflip, in_=d_sflip)
        nc.sync.dma_start(out=mL_sb, in_=d_mL)
        nc.sync.dma_start(out=mR_sb, in_=d_mR)
        nc.sync.dma_start(out=ones_sb, in_=d_ones)

        def load_wo():
            # deferred until after rope(0)'s swap DMAs: the FIFO DMA device
            # must not put bulk traffic in front of the latency-critical
            # swaps (wo isn't needed until oproj(0) in phase 1)
            for q2 in range(2):
                nc.sync.dma_start(
                    out=wo_sb[:, :, 1024 * q2:1024 * (q2 + 1)],
                    in_=d_wo[:, 1024 * q2:1024 * (q2 + 1)].rearrange(
                        "(a p) c -> p a c", p=128))

        # ================= stage emitters =================
        proj_state = {}

        def proj_alloc(g):
            assert g in xt_tiles, f"xt chunk {g} not prefetched"
            proj_state[g] = {
                "ps": {},
                "vd": vdp.tile([128, 4, 128], F32, tag="vd", name=f"vd_{g}"),
            }

        def proj_ps(g, i):
            # lazy per-feature psum: the 2-deep ring reuses a feature's bank
            # only after its rope has consumed it (k -> q0 -> q1 order)
            st = proj_state[g]
            if i not in st["ps"]:
                st["ps"][i] = qkvp.tile([128, 512], F32, tag="qkvps",
                                        name=f"qkvps_{g}_{i}")
            return st["ps"][i]

        NP_ = NH_T // 2   # DoubleRow passes (2 hidden tiles per pass)

        def mk_proj_feat(g, i, us):
            """One feature's (q0/q1/k) psum accumulation over DoubleRow
            passes `us`, three residual terms per pass."""
            def emit():
                if i == 2 and us[0] == 0:
                    proj_alloc(g)
                ps = proj_ps(g, i)
                xh_t, xl_t = xt_tiles[g]
                for u in us:
                    hsl = np.s_[:, 2 * u:2 * (u + 1)]
                    for t, (xa, wb) in enumerate(((xh_t, wh_sb), (xh_t, wl_sb),
                                                  (xl_t, wh_sb))):
                        nc.tensor.matmul(
                            ps[:],
                            wb[:, 2 * u:2 * (u + 1), 128 * i:128 * (i + 1)],
                            xa[hsl],
                            start=(u == 0 and t == 0),
                            stop=(u == NP_ - 1 and t == 2), perf_mode=DR)
            return emit

        def mk_proj_v(g, tt):
            # one token tile's full contraction as a contiguous start..stop
            # group: interleaved accumulation groups at different offsets of
            # the SAME psum bank produce wrong results on hardware
            def emit():
                st = proj_state[g]
                xh_t, xl_t = xt_tiles[g]
                for u in range(NP_):
                    for t, (xa, wb) in enumerate(((xh_t, wh_sb), (xh_t, wl_sb),
                                                  (xl_t, wh_sb))):
                        nc.tensor.matmul(
                            st["vd"][:, tt, :],
                            xa[:, 2 * u:2 * (u + 1), 128 * tt:128 * (tt + 1)],
                            wb[:, 2 * u:2 * (u + 1), 384:512],
                            start=(u == 0 and t == 0),
                            stop=(u == NP_ - 1 and t == 2), perf_mode=DR)
            return emit

        def proj_units_startup(g):
            """Chunk 0: k and q0 pass-major (follows the startup DMA arrival
            order, using both psum ring slots), then their ropes free k's
            bank for q1."""
            units = []

            def mk(u):
                def emit():
                    if u == 0:
                        proj_alloc(g)
                    xh_t, xl_t = xt_tiles[g]
                    for i in (2, 0):
                        ps = proj_ps(g, i)
                        for t, (xa, wb) in enumerate(((xh_t, wh_sb),
                                                      (xh_t, wl_sb),
                                                      (xl_t, wh_sb))):
                            nc.tensor.matmul(
                                ps[:],
                                wb[:, 2 * u:2 * (u + 1), 128 * i:128 * (i + 1)],
                                xa[:, 2 * u:2 * (u + 1), :],
                                start=(u == 0 and t == 0),
                                stop=(u == NP_ - 1 and t == 2), perf_mode=DR)
                return emit

            def late_loads():
                load_xt(1)
                load_wo()

            ru = rope_units(g, add_dve=True)   # DVE is idle at startup
            duos = [list(range(2 * q, 2 * q + 2)) for q in range(4)]
            units = [mk(u) for u in range(NP_)] + [ru[0], ru[2], late_loads]
            units += [mk_proj_v(g, tt) for tt in range(4)] + [ru[1]]
            units += [mk_proj_feat(g, 1, u2) for u2 in duos] + [ru[3]]
            return units

        def proj_units(g):
            """Chunk g's projection + rope as filler units, ordered so each
            feature's psum completes early and its rope follows immediately:
            k first (gates next phase's scores), then q0, V, q1."""
            ru = rope_units(g)
            duos = [list(range(2 * q, 2 * q + 2)) for q in range(4)]
            units = [mk_proj_feat(g, 2, u2) for u2 in duos] + [ru[0]]
            units += [mk_proj_feat(g, 0, u2) for u2 in duos] + [ru[2]]
            units += [mk_proj_v(g, tt) for tt in range(4)] + [ru[1]]
            units += [mk_proj_feat(g, 1, u2) for u2 in duos] + [ru[3]]
            return units

        def rope_units(g, add_dve=None):
            """V evac (Pool) + mRoPE for k/q0/q1 of chunk g as filler units,
            reading the projection psums (swapped halves via a bf16 scratch
            + SBUF->SBUF DMA), writing bf16 qkv_sb once. k first: it gates
            chunk g's scores."""
            tsl = np.s_[512 * g:512 * (g + 1)]
            xs = rp.tile([128, 3, 512], BF16, tag="xs", name=f"xs{g}")

            def mk_rope(t3):
                def emit():
                    st = proj_state[g]
                    psx = st["ps"][t3]
                    x = qkv_sb[:, t3, tsl]
                    xraw = rp.tile([128, 512], BF16, tag="xraw",
                                   name=f"xr_{g}_{t3}")
                    nc.vector.tensor_copy(xraw[:], psx[:])
                    nc.sync.dma_start(out=xs[0:64, t3, :], in_=xraw[64:128, :])
                    nc.sync.dma_start(out=xs[64:128, t3, :], in_=xraw[0:64, :])
                    t1 = rp.tile([128, 512], F32, tag="t1", name=f"t1_{g}_{t3}")
                    t2 = rp.tile([128, 512], F32, tag="t2", name=f"t2_{g}_{t3}")
                    # GPSIMD cannot touch PSUM: t1 (psum read) stays on DVE;
                    # t2 is SBUF-only so Pool takes it; the add returns to
                    # DVE (Pool's ~1.2us ops would serialize the k chain
                    # that gates the next phase's first scores)
                    nc.vector.tensor_mul(t1[:], psx[:], cdup[:, tsl])
                    nc.gpsimd.tensor_mul(t2[:], xs[:, t3, :], sflip[:, tsl])
                    on_dve = KNOBS["rope_add_dve"] if add_dve is None else add_dve
                    if on_dve:
                        nc.vector.tensor_add(x, t1[:], t2[:])
                    else:
                        nc.gpsimd.tensor_add(x, t1[:], t2[:])
                return emit

            def mk_vevac():
                def emit():
                    st = proj_state[g]
                    for tt in range(4):
                        nc.vector.tensor_copy(V_sb[:, 4 * g + tt, :],
                                              st["vd"][:, tt, :])
                return emit

            return [mk_rope(2), mk_vevac(), mk_rope(0), mk_rope(1)]

        def attn_steps(g):
            """Flat list of per-j-step emitters for both heads of chunk g.
            Scores run one step ahead of AV; head-0's denominator tail is
            emitted two steps into head 1 so its latency hides behind
            head-1 scores."""
            tsl = np.s_[512 * g:512 * (g + 1)]
            jmax = 4 * g + 4
            state = {}

            def head_alloc(h):
                state[h] = {
                    "po": avp.tile([128, 512], F32, tag="av", name=f"po{g}_{h}"),
                    "ra": racc.tile([128, 512], BF16, tag="ra", name=f"ra{g}_{h}"),
                    "Es": [None] * jmax,
                }

            # diagonal tiles (m = j-4g >= 0): queries [0:128m) can never see
            # this key tile, so scores/exp/AV/row-sum all trim to [128m:512]
            # and only the 128-wide boundary subtile needs the factored
            # triangular mask. The psum/E region left of 128m stays garbage
            # and is never read.
            def lo_of(j):
                return max(0, 128 * (j - 4 * g))

            def mk_av(h, j, stop):
                lo = lo_of(j)
                st = state[h]
                nc.tensor.matmul(st["po"][:, lo:], V_sb[:, j, :],
                                 st["Es"][j][:, lo:],
                                 start=(j == 0), stop=stop,
                                 skip_group_check=True)

            def mk_step(h, j):
                def emit():
                    if j == 0:
                        head_alloc(h)
                    st = state[h]
                    qc = qkv_sb[:, h, tsl]
                    m = j - 4 * g
                    lo = lo_of(j)
                    ps = spp.tile([128, 512], F32, tag="sp", name=f"s{g}_{h}_{j}")
                    nc.tensor.matmul(ps[:, lo:], qkv_sb[:, 2, 128 * j:128 * (j + 1)],
                                     qc[:, lo:], start=True, stop=(m < 0),
                                     skip_group_check=True)
                    if m >= 0:
                        nc.tensor.matmul(ps[:, lo:lo + 128], mL_sb[:], mR_sb[:],
                                         start=False, stop=True,
                                         skip_group_check=True)
                    E = ep.tile([128, 512], BF16, tag="e", name=f"e{g}_{h}_{j}")
                    st["Es"][j] = E
                    nc.scalar.activation(E[:, lo:], ps[:, lo:],
                                         mybir.ActivationFunctionType.Exp,
                                         scale=SCALE)
                    # row-sum partial accumulation on DVE (in-order engine:
                    # a single chain has the same throughput as two and a
                    # shorter tail)
                    if j == 0:
                        nc.vector.tensor_copy(st["ra"][:], E[:])
                    else:
                        nc.vector.tensor_add(st["ra"][:, lo:], st["ra"][:, lo:],
                                             E[:, lo:])
                    if j >= 1:
                        mk_av(h, j - 1, stop=False)
                return emit

            def mk_tail(h):
                def emit():
                    st = state[h]
                    mk_av(h, jmax - 1, stop=True)
                    # r broadcast across partitions via one all-ones matmul
                    pr = spp.tile([128, 512], F32, tag="sp", name=f"pr{g}_{h}")
                    nc.tensor.matmul(pr[:], ones_sb[:], st["ra"][:],
                                     start=True, stop=True)
                    rinv = rv.tile([128, 512], F32, tag="rv", name=f"rinv{g}_{h}")
                    nc.vector.reciprocal(rinv[:], pr[:])
                    nc.vector.tensor_mul(O_sb[:, h, tsl], st["po"][:], rinv[:])
                return emit

            steps = []
            for j in range(jmax):
                steps.append(mk_step(0, j))
                steps.append(mk_step(1, j))
            steps += [mk_tail(0), mk_tail(1)]
            return steps

        def oproj_units(g):
            """o_proj partial chunk: yT[:, tsl] = sum_h wo_h.T @ O_h, with
            psum evacuation rotated over DVE/ACT/Pool and a DMA per 4 tiles."""
            tsl = np.s_[512 * g:512 * (g + 1)]
            ybuf = yo.tile([128, NH_T, 512], BF16, tag="yo", name=f"yb{g}")

            # last chunk: finer DMA pieces so the final write drains with
            # the evacuations instead of after them
            per = 2 if g == NG - 1 else 4

            def mk(i):
                def emit():
                    py = spp.tile([128, 512], F32, tag="sp", name=f"y{g}_{i}")
                    for h in range(2):
                        nc.tensor.matmul(py[:], wo_sb[:, h, 128 * i:128 * (i + 1)],
                                         O_sb[:, h, tsl],
                                         start=(h == 0), stop=(h == 1))
                    # psum evacuation is DVE/ACT-only (GPSIMD cannot access
                    # PSUM); rotation split per KNOBS
                    m = KNOBS["ybuf_dve_mod"]
                    dve = (i % m == 2) if m else (i % 2 == 1)
                    if dve:
                        nc.vector.tensor_copy(ybuf[:, i, :], py[:])
                    else:
                        nc.scalar.copy(ybuf[:, i, :], py[:])
                    if i % per == per - 1:
                        lo = i - per + 1
                        nc.sync.dma_start(
                            out=d_yT[128 * lo:128 * (i + 1),
                                     tsl].rearrange("(a p) c -> p a c", p=128),
                            in_=ybuf[:, lo:i + 1, :])
                return emit

            return [mk(i) for i in range(NH_T)]

        def interleave(steps, fillers, lead=0):
            """Emit `lead` fillers up front (PE is in-order: a stalled step
            blocks everything emitted after it, so cover known step-0 latency
            with work emitted before it), then round-robin at ~1.5x rate so
            filler work front-loads and drains before the phase tail."""
            done = 0
            while done < min(lead, len(fillers)):
                fillers[done]()
                done += 1
            for si, s in enumerate(steps):
                s()
                want = max(done, (si + 1) * len(fillers) // len(steps))
                while done < min(want, len(fillers)):
                    fillers[done]()
                    done += 1
            while done < len(fillers):
                fillers[done]()
                done += 1

        # ================= schedule =================
        for u in proj_units_startup(0):
            u()
        for g in range(NG):
            # Filler assembly. Leads (emitted before attention step 0):
            # one ready-at-entry oproj unit + the next chunk's k projection
            # and k rope — the k rope chain gates the NEXT phase's first
            # scores, so it must start as early as possible. The xt
            # prefetch for g+2 is a filler placed after every rope swap
            # DMA of g+1 (the serialized DMA device is FIFO; 5.8us of xt
            # traffic in front of a swap stalls the next phase).
            ou = oproj_units(g - 1) if g > 0 else []
            pu = proj_units(g + 1) if g + 1 < NG else []
            head, tail = ou[:KNOBS["ou_head"]], ou[KNOBS["ou_head"]:]
            mixed = []
            for i in range(max(len(head), len(pu))):
                if KNOBS["ou_first"] and i < len(head):
                    mixed.append(head[i])
                if i < len(pu):
                    mixed.append(pu[i])
                if not KNOBS["ou_first"] and i < len(head):
                    mixed.append(head[i])
            fillers = mixed + tail
            interleave(attn_steps(g), fillers, lead=KNOBS["lead"])
            if g + 2 < NG:
                load_xt(g + 2)
        for u in oproj_units(NG - 1):
            u()

        if dbg:
            nc.sync.dma_start(out=d_qkv, in_=qkv_sb[:])
            nc.sync.dma_start(out=d_V, in_=V_sb[:])
            nc.sync.dma_start(out=d_O, in_=O_sb[:])

    nc.compile()
    return nc


_NC_CACHE = None


def _get_nc():
    global _NC_CACHE
    if _NC_CACHE is None:
        _NC_CACHE = _build()
    return _NC_CACHE


def _host_prep(positions, hidden_states, w_qkv, w_o):
    positions = np.asarray(positions, dtype=np.int32)
    hidden_states = np.asarray(hidden_states, dtype=np.float32)
    w_qkv = np.asarray(w_qkv, dtype=np.float32)
    w_o = np.asarray(w_o, dtype=np.float32)

    bf = ml_dtypes.bfloat16
    f8 = ml_dtypes.float8_e4m3

    # fp8 hi+lo residual split. Pre-scale by exact powers of two so both the
    # hi values and the ~3%-magnitude residuals sit in e4m3's normal range
    # (min normal 2^-6; unscaled w ~N(0,0.02) would be subnormal and the
    # residual would flush to zero). The x*8 * w*64 = 512x psum scale is
    # folded back out through the host rope tables (q/k path) and the wo
    # scaling (V path — softmax normalization is scale-invariant in V).
    SX, SW = 8.0, 64.0
    SINV = 1.0 / (SX * SW)

    def hilo(a, s):
        a = a * np.float32(s)
        hi = a.astype(f8)
        lo = (a - hi.astype(np.float32)).astype(f8)
        return np.ascontiguousarray(hi), np.ascontiguousarray(lo)

    xh, xl = hilo(np.ascontiguousarray(hidden_states.T), SX)
    # rope tables, host-computed: partition p holds rotation pair p%64 with
    # positions from ROW_MAP's t/h/w row. cdup = cos both halves;
    # sflip = [-sin; +sin] (so x*cdup + swap(x)*sflip rotates in place).
    pos_sel = positions[np.concatenate([ROW_MAP, ROW_MAP])].astype(np.float64)
    ang = pos_sel * np.concatenate([INVF, INVF]).astype(np.float64)[:, None]
    cdup = np.cos(ang) * SINV
    sflip = np.concatenate([-np.sin(ang[:64]), np.sin(ang[64:])], axis=0) * SINV
    # additive causal mask factors for the 128x128 diagonal boundary
    # subtile: invalid(dk, dq) = [dq < dk] = sum_p L[p,dk] * R[p,dq],
    #   L[p, dk] = [p <= dk],  R[p, dq] = -1e9 * [p == dq + 1]
    mask_l = (np.arange(128)[:, None] <= np.arange(128)[None, :]).astype(np.float32)
    mask_rb = np.zeros((128, 128), dtype=np.float32)
    mask_rb[np.arange(1, 128), np.arange(127)] = -1e9
    dq = np.arange(128)[None, :]
    dk = np.arange(128)[:, None]
    assert np.array_equal(mask_l.T @ mask_rb, np.where(dq < dk, -1e9, 0.0))
    ones = np.ones((128, 128), dtype=np.float32)

    q_size = N_HEADS * HD
    kv_size = N_KV * HD
    in_maps = []
    for c in range(NCORES):
        cols = [w_qkv[:, 2 * c * HD + PERM], w_qkv[:, (2 * c + 1) * HD + PERM]]
        kc = c // 2
        cols.append(w_qkv[:, q_size + kc * HD + PERM])
        cols.append(w_qkv[:, q_size + kv_size + kc * HD:q_size + kv_size + (kc + 1) * HD])
        wh, wl = hilo(np.concatenate(cols, axis=1), SW)
        wo_slice = np.ascontiguousarray(
            w_o[2 * c * HD:(2 * c + 2) * HD] * np.float32(SINV)).astype(bf)
        in_maps.append({
            "xh": xh, "xl": xl, "wh": wh, "wl": wl, "wo_slice": wo_slice,
            "cdup": np.ascontiguousarray(cdup).astype(bf),
            "sflip": np.ascontiguousarray(sflip).astype(bf),
            "mask_l": mask_l.astype(bf), "mask_rb": mask_rb.astype(bf),
            "ones": ones.astype(bf),
        })
    return in_maps


def kernel(positions, hidden_states, w_qkv, w_o):
    nc = _get_nc()
    in_maps = _host_prep(positions, hidden_states, w_qkv, w_o)
    # one retry: transient NRT/device errors (e.g. NRT_EXEC_UNIT_UNRECOVERABLE
    # from a wedged core) were observed to succeed on re-dispatch
    try:
        res = run_bass_kernel_spmd(nc, in_maps, core_ids=list(range(NCORES)))
    except Exception:
        import time
        time.sleep(2.0)
        res = run_bass_kernel_spmd(nc, in_maps, core_ids=list(range(NCORES)))
    yT = np.zeros((HIDDEN, T), dtype=np.float64)
    for c in range(NCORES):
        yT += np.asarray(res.results[c]["yT"], dtype=np.float64)
    return np.ascontiguousarray(yT.T).astype(np.float32)
